# revision 1
# baseline (speedup 1.0000x reference)
"""Trainium2 Bass kernel for nn_AGREE (group-member attention + predict MLP).

Data-parallel across 8 NeuronCores: B=16384 samples sharded 2048/core,
embedding tables + MLP weights replicated.

Per sample b:
  mem_e = user_table[member_ids[b]]            [50, 64]
  item_e = item_table[item_inputs[b]]          [64]
  h = relu(concat(mem_e, item_e) @ att_w1+b1)  [50, 16]
  scores = h @ att_w2 (+b2, softmax-invariant) [50]
  at_wt = softmax(scores masked to m <= member_lengths[b])
  g = at_wt @ mem_e + group_table[group_inputs[b]]
  y = sigmoid(relu([g*item, g, item] @ pred_w1 + pred_b1) @ pred_w2 + pred_b2)

Layout strategy (v2 — bf16 X-bar transposes, N=512 batched matmuls):
  - indirect DMA gathers member rows with inline f32->bf16 cast
  - member-pair tiles transposed via HWDGE DMA-transpose (bf16) straight
    into [128, 512] SBUF batches spanning 4 sample-tiles; zero PE transposes
  - attention MLP: pair matmuls K=128 (2 members) -> PSUM, item part (+b1)
    fused via accumulate + ACT relu-with-bias; scores via block-diag w2
  - masked softmax batched over 4 tiles; weighted member sum via fused
    scalar_tensor_tensor chain (bf16 in, f32 accum); predict MLP N=512.
"""

import sys

sys.path.insert(0, "/opt/trn_rl_repo")

import numpy as np

from concourse import bacc, bass, mybir
from concourse.tile import TileContext

NC = 8
B, M, D = 16384, 50, 64
BL = B // NC  # samples per core
P = 128
NT = BL // P  # sample tiles per core (16)
ST = 4        # sample-tiles per super-tile
NS = NT // ST  # super-tiles (4)
SW = ST * P   # samples per super-tile (512)
HID = 16
G8 = 8        # members per score group
NGRP = (M + G8 - 1) // G8  # 7 (last group has 2 members)
F32 = mybir.dt.float32
BF16 = mybir.dt.bfloat16
I32 = mybir.dt.int32

NUM_USERS, NUM_ITEMS, NUM_GROUPS = 100000, 50000, 20000

AF = mybir.ActivationFunctionType
OP = mybir.AluOpType
AX = mybir.AxisListType

_CACHE = {}


def build_nc():
    nc = bacc.Bacc()

    # --- data inputs (per-core shards), host-arranged tile-major:
    # plane[p, t...] = value for sample t*128+p ---
    ids_ext = nc.declare_dram_parameter("m_ids", [P, NT * M], I32, isOutput=False)
    item_ext = nc.declare_dram_parameter("i_idx", [P, NT], I32, isOutput=False)
    grp_ext = nc.declare_dram_parameter("g_idx", [P, NT], I32, isOutput=False)
    len_ext = nc.declare_dram_parameter("m_len", [P, NT], F32, isOutput=False)
    user_ext = nc.declare_dram_parameter("user_t", [NUM_USERS, D], F32, isOutput=False)
    itab_ext = nc.declare_dram_parameter("item_t", [NUM_ITEMS, D], F32, isOutput=False)
    gtab_ext = nc.declare_dram_parameter("group_t", [NUM_GROUPS, D], F32, isOutput=False)

    # --- static weight rearrangements (host-prepared, bf16) ---
    w1u2_ext = nc.declare_dram_parameter("w1u2", [P, 2 * HID], BF16, isOutput=False)
    w1i4_ext = nc.declare_dram_parameter("w1i4", [D, 4 * HID], BF16, isOutput=False)
    w2blk_ext = nc.declare_dram_parameter("w2blk", [P, G8], BF16, isOutput=False)
    pweg_ext = nc.declare_dram_parameter("pw_eg", [2 * D, 8], BF16, isOutput=False)
    pwit_ext = nc.declare_dram_parameter("pw_it", [D, 8], BF16, isOutput=False)
    pw2_ext = nc.declare_dram_parameter("pw2", [8, 1], BF16, isOutput=False)
    b1r_ext = nc.declare_dram_parameter("b1r", [D, 1], F32, isOutput=False)
    ident_ext = nc.declare_dram_parameter("ident", [P, P], BF16, isOutput=False)
    pb1_ext = nc.declare_dram_parameter("pb1", [8, 1], F32, isOutput=False)

    out_ext = nc.declare_dram_parameter("out", [BL, 1], F32, isOutput=True)

    with TileContext(nc) as tc:
        with (
            tc.tile_pool(name="const", bufs=1) as cn,
            tc.tile_pool(name="gbf", bufs=12) as gp,
            tc.tile_pool(name="sbuf", bufs=4) as sb,
            tc.tile_pool(name="small", bufs=6) as sm,
            tc.tile_pool(name="wide", bufs=2) as wd,
            tc.tile_pool(name="bigT", bufs=2) as bt,
            tc.tile_pool(name="psA", bufs=2, space="PSUM") as psA,
            tc.tile_pool(name="psC", bufs=1, space="PSUM") as psC,
            tc.tile_pool(name="psP", bufs=1, space="PSUM") as psP,
            tc.tile_pool(name="psT", bufs=2, space="PSUM") as psT,
        ):
            # ---- constants ----
            w1u2 = cn.tile([P, 2 * HID], BF16)
            nc.sync.dma_start(out=w1u2[:], in_=w1u2_ext[:])
            w1i4 = cn.tile([D, 4 * HID], BF16)
            nc.sync.dma_start(out=w1i4[:], in_=w1i4_ext[:])
            w2blk = cn.tile([P, G8], BF16)
            nc.sync.dma_start(out=w2blk[:], in_=w2blk_ext[:])
            pweg = cn.tile([2 * D, 8], BF16)
            nc.sync.dma_start(out=pweg[:], in_=pweg_ext[:])
            pwit = cn.tile([D, 8], BF16)
            nc.sync.dma_start(out=pwit[:], in_=pwit_ext[:])
            pw2 = cn.tile([8, 1], BF16)
            nc.sync.dma_start(out=pw2[:], in_=pw2_ext[:])
            b1r = cn.tile([D, 1], F32)
            nc.sync.dma_start(out=b1r[:], in_=b1r_ext[:])
            pb1 = cn.tile([8, 1], F32)
            nc.sync.dma_start(out=pb1[:], in_=pb1_ext[:])
            ident = cn.tile([P, P], BF16)
            nc.sync.dma_start(out=ident[:], in_=ident_ext[:])
            ids_all = cn.tile([P, NT * M], I32)
            nc.sync.dma_start(out=ids_all[:], in_=ids_ext[:])
            iidx_all = cn.tile([P, NT], I32)
            nc.sync.dma_start(out=iidx_all[:], in_=item_ext[:])
            gidx_all = cn.tile([P, NT], I32)
            nc.sync.dma_start(out=gidx_all[:], in_=grp_ext[:])
            len_all = cn.tile([P, NT], F32)
            nc.sync.dma_start(out=len_all[:], in_=len_ext[:])
            # device iota over members (0..49), f32
            iota_i = cn.tile([P, M], I32)
            nc.gpsimd.iota(iota_i[:], pattern=[[1, M]], base=0, channel_multiplier=0)
            iota_m = cn.tile([P, M], F32)
            nc.vector.tensor_copy(out=iota_m[:], in_=iota_i[:])
            # absorb the len-plane DMA into the DVE clock once
            warm = cn.tile([P, 1], F32)
            nc.vector.tensor_copy(out=warm[:], in_=len_all[:, 0:1])

            def issue_gathers(s):
                # bf16 inline-cast gathers for one super-tile
                g_bf, itgr, gr_f = [], [], []
                for tp in range(ST):
                    t = s * ST + tp
                    gb = gp.tile([P, M * D], BF16, tag="gbf", name=f"gb{t}")
                    nc.gpsimd.indirect_dma_start(
                        out=gb[:], out_offset=None, in_=user_ext[:],
                        in_offset=bass.IndirectOffsetOnAxis(
                            ap=ids_all[:, t * M : (t + 1) * M], axis=0),
                    )
                    g_bf.append(gb)
                    ig = sm.tile([P, 2 * D], BF16, tag="itgr", name=f"ig{t}")
                    nc.gpsimd.indirect_dma_start(
                        out=ig[:, :D], out_offset=None, in_=itab_ext[:],
                        in_offset=bass.IndirectOffsetOnAxis(
                            ap=iidx_all[:, t : t + 1], axis=0),
                    )
                    nc.gpsimd.indirect_dma_start(
                        out=ig[:, D:], out_offset=None, in_=gtab_ext[:],
                        in_offset=bass.IndirectOffsetOnAxis(
                            ap=gidx_all[:, t : t + 1], axis=0),
                    )
                    itgr.append(ig)
                return g_bf, itgr, gr_f

            pending = issue_gathers(0)
            for s in range(NS):
                g_bf, itgr, _ = pending
                # prefetch next super-tile's gathers before touching this one
                if s + 1 < NS:
                    pending = issue_gathers(s + 1)

                # ---------- X-bar transposes (item/group) ----------
                # itT[:, 128*tp+...]: rows 0:64 = item_e^T for tile tp
                itT = bt.tile([P, SW], BF16, tag="itT")
                for tp in range(ST):
                    tps = psT.tile([P, P], BF16, tag="tps")
                    nc.tensor.transpose(out=tps[:], in_=itgr[tp][:], identity=ident[:])
                    eng = nc.vector if tp % 2 == 0 else nc.scalar
                    if tp % 2 == 0:
                        nc.vector.tensor_copy(out=itT[:, tp * P : (tp + 1) * P], in_=tps[:])
                    else:
                        nc.scalar.activation(out=itT[:, tp * P : (tp + 1) * P], in_=tps[:], func=AF.Copy)

                # ---------- attention MLP ----------
                scps = psC.tile([P, ST * M], F32, tag="scps")
                for g in range(NGRP):
                    mg = min(G8, M - g * G8)
                    npr = (mg + 1) // 2  # member pairs in this group
                    rows = mg * HID
                    rows_a = min(rows, D)
                    rows_b = rows - rows_a
                    hp_a = psA.tile([D, 512], F32, tag="hpa")
                    hp_b = None
                    if rows_b > 0:
                        hp_b = psA.tile([D, 512], F32, tag="hpb")
                    pairTs = []
                    for j in range(npr):
                        q = 4 * g + j
                        pairT = sb.tile([P, SW], BF16, tag="pairT")
                        # two [128, 256] psum stages -> two copies per pair
                        for half in range(2):
                            tps = psT.tile([P, 2 * P], BF16, tag="tps")
                            for k in range(2):
                                tp = 2 * half + k
                                nc.tensor.transpose(
                                    out=tps[:, k * P : (k + 1) * P],
                                    in_=g_bf[tp][:, 2 * q * D : 2 * (q + 1) * D],
                                    identity=ident[:],
                                )
                            if (j + half) % 2 == 0:
                                nc.vector.tensor_copy(
                                    out=pairT[:, half * 2 * P : (half + 1) * 2 * P],
                                    in_=tps[:])
                            else:
                                nc.scalar.activation(
                                    out=pairT[:, half * 2 * P : (half + 1) * 2 * P],
                                    in_=tps[:], func=AF.Copy)
                        pairTs.append(pairT)
                    for j in range(npr):
                        hp_t = hp_a if j < 2 else hp_b
                        off = 2 * HID * (j % 2)
                        nc.tensor.matmul(
                            out=hp_t[off : off + 2 * HID, :SW],
                            lhsT=w1u2[:], rhs=pairTs[j][:],
                            start=True, stop=False, skip_group_check=True,
                        )
                    # item part accumulates over the opened regions, closes
                    nc.tensor.matmul(
                        out=hp_a[:rows_a, :SW], lhsT=w1i4[:, :rows_a],
                        rhs=itT[:D, :], start=False, stop=True,
                        skip_group_check=True,
                    )
                    if rows_b > 0:
                        nc.tensor.matmul(
                            out=hp_b[:rows_b, :SW], lhsT=w1i4[:, :rows_b],
                            rhs=itT[:D, :], start=False, stop=True,
                            skip_group_check=True,
                        )
                    # relu(x + b1) on ACT, straight to bf16
                    ht4 = sb.tile([P, SW], BF16, tag="ht4")
                    nc.scalar.activation(
                        out=ht4[:rows_a, :], in_=hp_a[:rows_a, :SW],
                        func=AF.Relu, bias=b1r[:rows_a, :],
                    )
                    if rows_b > 0:
                        nc.scalar.activation(
                            out=ht4[D : D + rows_b, :], in_=hp_b[:rows_b, :SW],
                            func=AF.Relu, bias=b1r[:rows_b, :],
                        )
                    # scores for this group land [128 samples, mg] per tile
                    for tp in range(ST):
                        nc.tensor.matmul(
                            out=scps[:, tp * M + g * G8 : tp * M + g * G8 + mg],
                            lhsT=ht4[:rows, tp * P : (tp + 1) * P],
                            rhs=w2blk[:rows, :mg],
                            start=True, stop=True,
                        )

                # ---------- masked softmax (batched over 4 tiles) ----------
                msk = sb.tile([P, ST * M], F32, tag="msk")
                for tp in range(ST):
                    nc.vector.tensor_scalar(
                        out=msk[:, tp * M : (tp + 1) * M], in0=iota_m[:],
                        scalar1=len_all[:, s * ST + tp : s * ST + tp + 1],
                        scalar2=None, op0=OP.is_le,
                    )
                scm = sb.tile([P, ST * M], F32, tag="scm")
                nc.vector.scalar_tensor_tensor(
                    out=scm[:], in0=scps[:], scalar=30.0, in1=msk[:],
                    op0=OP.add, op1=OP.mult,
                )
                mx4 = sb.tile([P, ST], F32, tag="mx4")
                nc.vector.tensor_reduce(
                    out=mx4[:], in_=scm[:].rearrange("p (t m) -> p t m", m=M),
                    axis=AX.X, op=OP.max,
                )
                ein = sb.tile([P, ST * M], F32, tag="ein")
                nc.vector.tensor_tensor(
                    out=ein[:].rearrange("p (t m) -> p t m", m=M),
                    in0=scm[:].rearrange("p (t m) -> p t m", m=M),
                    in1=mx4[:].rearrange("p (t one) -> p t one", one=1)
                    .to_broadcast([P, ST, M]),
                    op=OP.subtract,
                )
                e4 = sb.tile([P, ST * M], F32, tag="e4")
                nc.scalar.activation(out=e4[:], in_=ein[:], func=AF.Exp)
                z4 = sb.tile([P, ST], F32, tag="z4")
                nc.vector.tensor_reduce(
                    out=z4[:], in_=e4[:].rearrange("p (t m) -> p t m", m=M),
                    axis=AX.X, op=OP.add,
                )
                rz4 = sb.tile([P, ST], F32, tag="rz4")
                nc.vector.reciprocal(out=rz4[:], in_=z4[:])


                # ---------- weighted member sum + g, per tile ----------
                e_bf = sb.tile([P, ST * M], BF16, tag="ebf")
                nc.vector.tensor_copy(out=e_bf[:], in_=e4[:])
                egT = bt.tile([P, SW], BF16, tag="egT")
                for tp in range(ST):
                    # weighted rows: all-bf16 product, then reduce over members
                    prod = wd.tile([P, M * D], BF16, tag="prod")
                    nc.vector.tensor_tensor(
                        out=prod[:].rearrange("p (m d) -> p m d", d=D),
                        in0=g_bf[tp][:].rearrange("p (m d) -> p m d", d=D),
                        in1=e_bf[:, tp * M : (tp + 1) * M]
                        .rearrange("p (m one) -> p m one", one=1)
                        .to_broadcast([P, M, D]),
                        op=OP.mult,
                    )
                    # log-tree pairwise adds over member blocks (contiguous reads)
                    tre = wd.tile([P, 25 * D], F32, tag="tre")
                    nc.vector.tensor_tensor(
                        out=tre[:].rearrange("p (m d) -> p m d", d=D),
                        in0=prod[:].rearrange(
                            "p (m two d) -> p m two d", two=2, d=D)[:, :, 0, :],
                        in1=prod[:].rearrange(
                            "p (m two d) -> p m two d", two=2, d=D)[:, :, 1, :],
                        op=OP.add,
                    )
                    n = 25
                    buf = tre
                    while n > 1:
                        k = n // 2
                        odd = n - 2 * k
                        nxt_t = wd.tile([P, (k + odd) * D], F32, tag=f"tr{n}")
                        nc.vector.tensor_tensor(
                            out=nxt_t[:, : k * D].rearrange("p (m d) -> p m d", d=D),
                            in0=buf[:, : 2 * k * D].rearrange(
                                "p (m two d) -> p m two d", two=2, d=D)[:, :, 0, :],
                            in1=buf[:, : 2 * k * D].rearrange(
                                "p (m two d) -> p m two d", two=2, d=D)[:, :, 1, :],
                            op=OP.add,
                        )
                        if odd:
                            nc.vector.tensor_copy(
                                out=nxt_t[:, k * D :], in_=buf[:, 2 * k * D :])
                        buf = nxt_t
                        n = k + odd
                    acc = buf
                    # eg rows: [elem (0:64) | g (64:128)] bf16, then transpose
                    eg = sm.tile([P, 2 * D], BF16, tag="eg")
                    nc.vector.scalar_tensor_tensor(
                        out=eg[:, D:], in0=acc[:],
                        scalar=rz4[:, tp : tp + 1], in1=itgr[tp][:, D:],
                        op0=OP.mult, op1=OP.add,
                    )
                    nc.vector.tensor_tensor(
                        out=eg[:, :D], in0=eg[:, D:], in1=itgr[tp][:, :D],
                        op=OP.mult,
                    )
                    tps2 = psT.tile([P, P], BF16, tag="tps")
                    nc.tensor.transpose(out=tps2[:], in_=eg[:], identity=ident[:])
                    nc.vector.tensor_copy(
                        out=egT[:, tp * P : (tp + 1) * P], in_=tps2[:])

                # ---------- predict MLP (N=512) ----------
                pp = psP.tile([8, 512], F32, tag="pp")
                nc.tensor.matmul(
                    out=pp[:, :SW], lhsT=pweg[:], rhs=egT[:],
                    start=True, stop=False, skip_group_check=True,
                )
                nc.tensor.matmul(
                    out=pp[:, :SW], lhsT=pwit[:], rhs=itT[:D, :],
                    start=False, stop=True, skip_group_check=True,
                )
                ph = sb.tile([8, SW], BF16, tag="ph")
                nc.scalar.activation(
                    out=ph[:], in_=pp[:, :SW], func=AF.Relu, bias=pb1[:]
                )
                y_ps = psP.tile([1, SW], F32, tag="pp")
                nc.tensor.matmul(
                    out=y_ps[:], lhsT=pw2[:], rhs=ph[:], start=True, stop=True
                )
                y_sb = sb.tile([1, SW], F32, tag="ysb")
                nc.scalar.activation(out=y_sb[:], in_=y_ps[:], func=AF.Sigmoid)
                nc.sync.dma_start(
                    out=out_ext[s * SW : (s + 1) * SW, :], in_=y_sb[:]
                )

    nc.compile()
    return nc


def _statics(att_w1, att_b1, att_w2, pred_w1, pred_b1, pred_w2, pred_b2):
    import ml_dtypes

    bf = ml_dtypes.bfloat16
    # member-pair projection: blockdiag(W1u, W1u)
    w1u2 = np.zeros((P, 2 * HID), dtype=np.float32)
    w1u2[:D, :HID] = att_w1[:D, :]
    w1u2[D:, HID:] = att_w1[:D, :]
    # item-part weights tiled over 4 member blocks (no bias row; b1 via ACT)
    w1i4 = np.tile(att_w1[D:, :], (1, 4))
    # block-diagonal w2 for scores
    w2blk = np.zeros((P, G8), dtype=np.float32)
    for j in range(G8):
        w2blk[j * HID : (j + 1) * HID, j] = att_w2[:, 0]
    # predict: rows [elem (pred_w1[0:64]) ; g (pred_w1[64:128])]
    pw_eg = pred_w1[: 2 * D, :]
    pw_it = pred_w1[2 * D :, :]
    b1r = np.tile(att_b1, 4).reshape(D, 1).astype(np.float32)
    return dict(
        w1u2=w1u2.astype(bf), w1i4=w1i4.astype(bf), w2blk=w2blk.astype(bf),
        pw_eg=pw_eg.astype(bf), pw_it=pw_it.astype(bf),
        pw2=pred_w2.astype(bf), b1r=b1r, ident=np.eye(P, dtype=np.float32).astype(bf),
        pb1=pred_b1.reshape(8, 1).astype(np.float32),
    )


def make_in_maps(**inputs):
    st = _statics(
        np.asarray(inputs["att_w1"], np.float32),
        np.asarray(inputs["att_b1"], np.float32),
        np.asarray(inputs["att_w2"], np.float32),
        np.asarray(inputs["pred_w1"], np.float32),
        np.asarray(inputs["pred_b1"], np.float32),
        np.asarray(inputs["pred_w2"], np.float32),
        np.asarray(inputs["pred_b2"], np.float32),
    )

    def tile_major(x):
        # [BL(, k)] -> [P, NT(*k)]: column-block t holds samples t*128..t*128+127
        x = x.reshape(NT, P, -1)
        return np.ascontiguousarray(x.transpose(1, 0, 2).reshape(P, -1))

    m_ids = np.asarray(inputs["member_ids"], np.int32).reshape(NC, BL, M)
    i_idx = np.asarray(inputs["item_inputs"], np.int32).reshape(NC, BL)
    g_idx = np.asarray(inputs["group_inputs"], np.int32).reshape(NC, BL)
    m_len = np.asarray(inputs["member_lengths"], np.float32).reshape(NC, BL)
    user_t = np.ascontiguousarray(np.asarray(inputs["user_table"], np.float32))
    item_t = np.ascontiguousarray(np.asarray(inputs["item_table"], np.float32))
    group_t = np.ascontiguousarray(np.asarray(inputs["group_table"], np.float32))

    in_maps = []
    for c in range(NC):
        in_maps.append(
            {
                "m_ids": tile_major(m_ids[c]),
                "i_idx": tile_major(i_idx[c]),
                "g_idx": tile_major(g_idx[c]),
                "m_len": tile_major(m_len[c]),
                "user_t": user_t,
                "item_t": item_t,
                "group_t": group_t,
                "w1u2": st["w1u2"],
                "w1i4": st["w1i4"],
                "w2blk": st["w2blk"],
                "pw_eg": st["pw_eg"],
                "pw_it": st["pw_it"],
                "pw2": st["pw2"],
                "b1r": st["b1r"],
                "pb1": st["pb1"],
                "ident": st["ident"],
            }
        )
    return in_maps


def get_nc():
    if "nc" not in _CACHE:
        _CACHE["nc"] = build_nc()
    return _CACHE["nc"]


def kernel(**inputs):
    from concourse.bass_utils import run_bass_kernel_spmd

    nc = get_nc()
    in_maps = make_in_maps(**inputs)
    res = run_bass_kernel_spmd(nc, in_maps, core_ids=list(range(NC)))
    return np.concatenate([r["out"] for r in res.results], axis=0)



# revision 7
# speedup vs baseline: 2.4213x; 2.4213x over previous
"""Trainium2 Bass kernel for nn_AGREE (group-member attention + predict MLP).

Data-parallel across 8 NeuronCores: B=16384 samples sharded 2048/core,
embedding tables + MLP weights replicated (tables pre-cast to bf16 on host).

v3 strategy (vs v2 baseline):
  - host sorts each core's samples by member_lengths; super-tile s only
    processes W_s = (max valid members in that super-tile) member slots,
    cutting gather/transpose/matmul/reduce work to ~60%.  W_s are known
    only at run time, so build_nc() is parameterized and compiled lazily
    inside kernel() (compile time is not execution time).
  - one merged indirect gather per super-tile (member rows, bf16 table),
    plus one item + one group gather; prefetched 2 super-tiles ahead.
  - attention: PE transposes member pairs -> pair matmuls (K=128, N=512)
    into a single [128,512] PSUM tile per 8-member group; item part via
    one w1i8 [64,128] matmul; relu+bias on ACT; block-diag w2 scores.
  - masked softmax batched over the 4 tiles of a super-tile; weighted
    member sum via bf16 pairwise log-tree on DVE.
  - predict MLP N=512; its PE work (eg transposes + matmuls) is emitted
    one super-tile late so it never stalls the PE behind the DVE tail.
"""

import sys

sys.path.insert(0, "/opt/trn_rl_repo")

import numpy as np

from concourse import bacc, bass, mybir
from concourse.tile import TileContext

NC = 8
B, M, D = 16384, 50, 64
BL = B // NC  # samples per core
P = 128
NT = BL // P   # sample tiles per core (16)
ST = 4         # sample-tiles per super-tile
NS = NT // ST  # super-tiles (4)
SW = ST * P    # samples per super-tile (512)
HID = 16
G8 = 8         # members per score group
F32 = mybir.dt.float32
BF16 = mybir.dt.bfloat16
I32 = mybir.dt.int32

NUM_USERS, NUM_ITEMS, NUM_GROUPS = 100000, 50000, 20000

AF = mybir.ActivationFunctionType
OP = mybir.AluOpType
AX = mybir.AxisListType

_CACHE = {}


def build_nc(widths):
    """widths: tuple of NS even ints (2..50) = member slots per super-tile."""
    assert len(widths) == NS and all(2 <= w <= M and w % 2 == 0 for w in widths)
    offs = [0]
    for w in widths:
        offs.append(offs[-1] + ST * w)
    IDC = offs[-1]  # total id columns

    nc = bacc.Bacc()

    # --- per-core data (host-sorted by length, tile-major planes) ---
    ids_ext = nc.declare_dram_parameter("m_ids", [P, IDC], I32, isOutput=False)
    item_ext = nc.declare_dram_parameter("i_idx", [P, NT], I32, isOutput=False)
    grp_ext = nc.declare_dram_parameter("g_idx", [P, NT], I32, isOutput=False)
    len_ext = nc.declare_dram_parameter("m_len", [P, NT], F32, isOutput=False)
    user_ext = nc.declare_dram_parameter("user_t", [NUM_USERS, D], BF16, isOutput=False)
    itab_ext = nc.declare_dram_parameter("item_t", [NUM_ITEMS, D], BF16, isOutput=False)
    gtab_ext = nc.declare_dram_parameter("group_t", [NUM_GROUPS, D], BF16, isOutput=False)

    # --- static weights (host-prepared) ---
    w1u2_ext = nc.declare_dram_parameter("w1u2", [P, 2 * HID], BF16, isOutput=False)
    w1i8_ext = nc.declare_dram_parameter("w1i8", [D, P], BF16, isOutput=False)
    w2blk_ext = nc.declare_dram_parameter("w2blk", [P, G8], BF16, isOutput=False)
    pweg_ext = nc.declare_dram_parameter("pw_eg", [2 * D, 8], BF16, isOutput=False)
    pwit_ext = nc.declare_dram_parameter("pw_it", [D, 8], BF16, isOutput=False)
    pw2_ext = nc.declare_dram_parameter("pw2", [8, 1], BF16, isOutput=False)
    b1r8_ext = nc.declare_dram_parameter("b1r8", [P, 1], F32, isOutput=False)
    ident_ext = nc.declare_dram_parameter("ident", [P, P], BF16, isOutput=False)
    pb1_ext = nc.declare_dram_parameter("pb1", [8, 1], F32, isOutput=False)
    pb2_ext = nc.declare_dram_parameter("pb2", [1, 1], F32, isOutput=False)

    out_ext = nc.declare_dram_parameter("out", [BL, 1], F32, isOutput=True)

    with TileContext(nc) as tc:
        with (
            tc.tile_pool(name="const", bufs=1) as cn,
            tc.tile_pool(name="gath", bufs=3) as gp,
            tc.tile_pool(name="sbuf", bufs=4) as sb,
            tc.tile_pool(name="soft", bufs=2) as sf,
            tc.tile_pool(name="wide", bufs=2) as wd,
            tc.tile_pool(name="psA", bufs=2, space="PSUM") as psA,
            tc.tile_pool(name="psC", bufs=2, space="PSUM") as psC,
            tc.tile_pool(name="psP", bufs=1, space="PSUM") as psP,
            tc.tile_pool(name="psT", bufs=3, space="PSUM") as psT,
        ):
            # ---- index planes first (gathers depend on them) ----
            ids_all = cn.tile([P, IDC], I32)
            nc.sync.dma_start(out=ids_all[:], in_=ids_ext[:])
            iidx_all = cn.tile([P, NT], I32)
            nc.sync.dma_start(out=iidx_all[:], in_=item_ext[:])
            gidx_all = cn.tile([P, NT], I32)
            nc.sync.dma_start(out=gidx_all[:], in_=grp_ext[:])

            def issue_gathers(s):
                w = widths[s]
                gw = gp.tile([P, ST * w * D], BF16, tag="gw", name=f"gw{s}")
                nc.gpsimd.indirect_dma_start(
                    out=gw[:], out_offset=None, in_=user_ext[:],
                    in_offset=bass.IndirectOffsetOnAxis(
                        ap=ids_all[:, offs[s] : offs[s + 1]], axis=0),
                )
                it4 = gp.tile([P, ST * D], BF16, tag="it4", name=f"it{s}")
                nc.gpsimd.indirect_dma_start(
                    out=it4[:], out_offset=None, in_=itab_ext[:],
                    in_offset=bass.IndirectOffsetOnAxis(
                        ap=iidx_all[:, s * ST : (s + 1) * ST], axis=0),
                )
                gr4 = gp.tile([P, ST * D], BF16, tag="gr4", name=f"gr{s}")
                nc.gpsimd.indirect_dma_start(
                    out=gr4[:], out_offset=None, in_=gtab_ext[:],
                    in_offset=bass.IndirectOffsetOnAxis(
                        ap=gidx_all[:, s * ST : (s + 1) * ST], axis=0),
                )
                return gw, it4, gr4

            pend = [issue_gathers(0), issue_gathers(1)]

            # ---- remaining constants (overlap with first gathers) ----
            len_all = cn.tile([P, NT], F32)
            nc.sync.dma_start(out=len_all[:], in_=len_ext[:])
            w1u2 = cn.tile([P, 2 * HID], BF16)
            nc.sync.dma_start(out=w1u2[:], in_=w1u2_ext[:])
            w1i8 = cn.tile([D, P], BF16)
            nc.sync.dma_start(out=w1i8[:], in_=w1i8_ext[:])
            w2blk = cn.tile([P, G8], BF16)
            nc.sync.dma_start(out=w2blk[:], in_=w2blk_ext[:])
            pweg = cn.tile([2 * D, 8], BF16)
            nc.sync.dma_start(out=pweg[:], in_=pweg_ext[:])
            pwit = cn.tile([D, 8], BF16)
            nc.sync.dma_start(out=pwit[:], in_=pwit_ext[:])
            pw2 = cn.tile([8, 1], BF16)
            nc.sync.dma_start(out=pw2[:], in_=pw2_ext[:])
            b1r8 = cn.tile([P, 1], F32)
            nc.sync.dma_start(out=b1r8[:], in_=b1r8_ext[:])
            pb1 = cn.tile([8, 1], F32)
            nc.sync.dma_start(out=pb1[:], in_=pb1_ext[:])
            pb2 = cn.tile([1, 1], F32)
            nc.sync.dma_start(out=pb2[:], in_=pb2_ext[:])
            ident = cn.tile([P, P], BF16)
            nc.sync.dma_start(out=ident[:], in_=ident_ext[:])
            iota_i = cn.tile([P, M], I32)
            nc.gpsimd.iota(iota_i[:], pattern=[[1, M]], base=0, channel_multiplier=0)
            iota_m = cn.tile([P, M], F32)
            nc.vector.tensor_copy(out=iota_m[:], in_=iota_i[:])
            # pre-warm ACT function tables off the critical path
            wrm = cn.tile([P, 4], F32)
            nc.vector.tensor_copy(out=wrm[:, 0:1], in_=len_all[:, 0:1])
            nc.scalar.activation(out=wrm[:, 1:2], in_=wrm[:, 0:1], func=AF.Relu)
            nc.scalar.activation(out=wrm[:, 2:3], in_=wrm[:, 0:1], func=AF.Exp)
            nc.scalar.activation(out=wrm[:, 3:4], in_=wrm[:, 0:1], func=AF.Sigmoid)

            def stage_a(s, gw, it4, gr4):
                """Attention + softmax + weighted sum; returns (egT-parts)."""
                w = widths[s]
                ng = (w + G8 - 1) // G8

                # item transposes -> itT [64 rows d, 512 samples]
                itT = sb.tile([D, SW], BF16, tag="itT", bufs=2)
                for h2 in range(2):
                    tpi = psT.tile([D, 2 * P], BF16, tag="tps")
                    for k in range(2):
                        tp = 2 * h2 + k
                        nc.tensor.transpose(
                            out=tpi[:, k * P : (k + 1) * P],
                            in_=it4[:, tp * D : (tp + 1) * D],
                            identity=ident[:])
                    nc.vector.tensor_copy(
                        out=itT[:, h2 * 2 * P : (h2 + 1) * 2 * P], in_=tpi[:])

                # ---------- attention MLP ----------
                scps = psC.tile([P, ST * w], F32, tag="scps")
                cpi = 0
                for g in range(ng):
                    mg = min(G8, w - g * G8)
                    npr = mg // 2
                    rows = mg * HID
                    hp = psA.tile([P, SW], F32, tag="hp")
                    pairTs = []
                    for j in range(npr):
                        m0 = g * G8 + 2 * j
                        pairT = sb.tile([P, SW], BF16, tag="pairT")
                        tps = psT.tile([P, SW], BF16, tag="tps")
                        for tp in range(ST):
                            base = (tp * w + m0) * D
                            nc.tensor.transpose(
                                out=tps[:, tp * P : (tp + 1) * P],
                                in_=gw[:, base : base + 2 * D],
                                identity=ident[:],
                            )
                        # 2/3 of copies on ACT, 1/3 on DVE
                        for half in range(2):
                            if cpi % 3 == 2:
                                nc.vector.tensor_copy(
                                    out=pairT[:, half * 2 * P : (half + 1) * 2 * P],
                                    in_=tps[:, half * 2 * P : (half + 1) * 2 * P])
                            else:
                                nc.scalar.activation(
                                    out=pairT[:, half * 2 * P : (half + 1) * 2 * P],
                                    in_=tps[:, half * 2 * P : (half + 1) * 2 * P],
                                    func=AF.Copy)
                            cpi += 1
                        pairTs.append(pairT)
                    for j in range(npr):
                        nc.tensor.matmul(
                            out=hp[32 * j : 32 * (j + 1), :SW],
                            lhsT=w1u2[:], rhs=pairTs[j][:],
                            start=True, stop=False, skip_group_check=True,
                            tile_position=(0, 32 * j),
                        )
                    nc.tensor.matmul(
                        out=hp[:rows, :SW], lhsT=w1i8[:, :rows], rhs=itT[:],
                        start=False, stop=True, skip_group_check=True,
                    )
                    ht = sb.tile([P, SW], BF16, tag="ht", bufs=3)
                    nc.scalar.activation(
                        out=ht[:rows, :], in_=hp[:rows, :SW],
                        func=AF.Relu, bias=b1r8[:rows, :],
                    )
                    for tp in range(ST):
                        nc.tensor.matmul(
                            out=scps[:, tp * w + g * G8 : tp * w + g * G8 + mg],
                            lhsT=ht[:rows, tp * P : (tp + 1) * P],
                            rhs=w2blk[:rows, :mg],
                            start=True, stop=True,
                        )

                # ---------- masked softmax (batched over 4 tiles) ----------
                msk = sf.tile([P, ST * w], F32, tag="msk")
                for tp in range(ST):
                    nc.vector.tensor_scalar(
                        out=msk[:, tp * w : (tp + 1) * w], in0=iota_m[:, :w],
                        scalar1=len_all[:, s * ST + tp : s * ST + tp + 1],
                        scalar2=None, op0=OP.is_le,
                    )
                scm = sf.tile([P, ST * w], F32, tag="scm")
                nc.vector.scalar_tensor_tensor(
                    out=scm[:], in0=scps[:], scalar=30.0, in1=msk[:],
                    op0=OP.add, op1=OP.mult,
                )
                mx4 = sf.tile([P, ST], F32, tag="mx4")
                nc.vector.tensor_reduce(
                    out=mx4[:], in_=scm[:].rearrange("p (t m) -> p t m", m=w),
                    axis=AX.X, op=OP.max,
                )
                ein = sf.tile([P, ST * w], F32, tag="ein")
                nc.vector.tensor_tensor(
                    out=ein[:].rearrange("p (t m) -> p t m", m=w),
                    in0=scm[:].rearrange("p (t m) -> p t m", m=w),
                    in1=mx4[:].rearrange("p (t one) -> p t one", one=1)
                    .to_broadcast([P, ST, w]),
                    op=OP.subtract,
                )
                e4 = sf.tile([P, ST * w], F32, tag="e4")
                nc.scalar.activation(out=e4[:], in_=ein[:], func=AF.Exp)
                z4 = sf.tile([P, ST], F32, tag="z4")
                nc.vector.tensor_reduce(
                    out=z4[:], in_=e4[:].rearrange("p (t m) -> p t m", m=w),
                    axis=AX.X, op=OP.add,
                )
                rz4 = sf.tile([P, ST], F32, tag="rz4")
                nc.vector.reciprocal(out=rz4[:], in_=z4[:])
                e_bf = sf.tile([P, ST * w], BF16, tag="ebf")
                nc.vector.tensor_copy(out=e_bf[:], in_=e4[:])

                # ---------- weighted member sum + eg rows, per tile ----------
                egs = []
                for tp in range(ST):
                    prod = wd.tile([P, w * D], BF16, tag="prod")
                    nc.vector.tensor_tensor(
                        out=prod[:].rearrange("p (m d) -> p m d", d=D),
                        in0=gw[:, tp * w * D : (tp + 1) * w * D]
                        .rearrange("p (m d) -> p m d", d=D),
                        in1=e_bf[:, tp * w : (tp + 1) * w]
                        .rearrange("p (m one) -> p m one", one=1)
                        .to_broadcast([P, w, D]),
                        op=OP.mult,
                    )
                    # pairwise log-tree reduce over members (bf16)
                    n = w // 2
                    buf = wd.tile([P, n * D], BF16, tag="trA")
                    nc.vector.tensor_tensor(
                        out=buf[:].rearrange("p (m d) -> p m d", d=D),
                        in0=prod[:].rearrange(
                            "p (m two d) -> p m two d", two=2, d=D)[:, :, 0, :],
                        in1=prod[:].rearrange(
                            "p (m two d) -> p m two d", two=2, d=D)[:, :, 1, :],
                        op=OP.add,
                    )
                    ab = 0
                    while n > 1:
                        k = n // 2
                        odd = n - 2 * k
                        nxt = wd.tile(
                            [P, (k + odd) * D], BF16, tag=f"tr{'BA'[ab]}")
                        ab ^= 1
                        nc.vector.tensor_tensor(
                            out=nxt[:, : k * D].rearrange("p (m d) -> p m d", d=D),
                            in0=buf[:, : 2 * k * D].rearrange(
                                "p (m two d) -> p m two d", two=2, d=D)[:, :, 0, :],
                            in1=buf[:, : 2 * k * D].rearrange(
                                "p (m two d) -> p m two d", two=2, d=D)[:, :, 1, :],
                            op=OP.add,
                        )
                        if odd:
                            nc.vector.tensor_copy(
                                out=nxt[:, k * D :], in_=buf[:, 2 * k * D :])
                        buf = nxt
                        n = k + odd
                    # eg rows: [elem (0:64) | g (64:128)] bf16
                    eg = sb.tile([P, 2 * D], BF16, tag="eg", bufs=8)
                    nc.vector.scalar_tensor_tensor(
                        out=eg[:, D:], in0=buf[:],
                        scalar=rz4[:, tp : tp + 1],
                        in1=gr4[:, tp * D : (tp + 1) * D],
                        op0=OP.mult, op1=OP.add,
                    )
                    nc.vector.tensor_tensor(
                        out=eg[:, :D], in0=eg[:, D:],
                        in1=it4[:, tp * D : (tp + 1) * D],
                        op=OP.mult,
                    )
                    egs.append(eg)
                return itT, egs

            def stage_b(s, itT, egs):
                """Predict MLP for super-tile s (PE work deferred one ST)."""
                egT = sb.tile([P, SW], BF16, tag="egT", bufs=2)
                for tp in range(ST):
                    tps2 = psT.tile([P, P], BF16, tag="tps")
                    nc.tensor.transpose(
                        out=tps2[:], in_=egs[tp][:], identity=ident[:])
                    nc.vector.tensor_copy(
                        out=egT[:, tp * P : (tp + 1) * P], in_=tps2[:])
                pp = psP.tile([8, SW], F32, tag="pp")
                nc.tensor.matmul(
                    out=pp[:, :SW], lhsT=pweg[:], rhs=egT[:],
                    start=True, stop=False, skip_group_check=True,
                )
                nc.tensor.matmul(
                    out=pp[:, :SW], lhsT=pwit[:], rhs=itT[:],
                    start=False, stop=True, skip_group_check=True,
                )
                ph = sb.tile([8, SW], BF16, tag="ph", bufs=2)
                nc.scalar.activation(
                    out=ph[:], in_=pp[:, :SW], func=AF.Relu, bias=pb1[:]
                )
                y_ps = psP.tile([1, SW], F32, tag="pp")
                nc.tensor.matmul(
                    out=y_ps[:], lhsT=pw2[:], rhs=ph[:], start=True, stop=True
                )
                y_sb = sb.tile([1, SW], F32, tag="ysb", bufs=2)
                nc.scalar.activation(
                    out=y_sb[:], in_=y_ps[:], func=AF.Sigmoid, bias=pb2[:])
                nc.sync.dma_start(
                    out=out_ext[s * SW : (s + 1) * SW, :], in_=y_sb[:]
                )

            held = None
            for s in range(NS):
                gw, it4, gr4 = pend.pop(0)
                if s + 2 < NS:
                    pend.append(issue_gathers(s + 2))
                res = stage_a(s, gw, it4, gr4)
                if held is not None:
                    stage_b(s - 1, *held)
                held = res
            stage_b(NS - 1, *held)

    nc.compile()
    return nc


def _statics(att_w1, att_b1, att_w2, pred_w1, pred_b1, pred_w2, pred_b2):
    import ml_dtypes

    bf = ml_dtypes.bfloat16
    # member-pair projection: blockdiag(W1u, W1u)
    w1u2 = np.zeros((P, 2 * HID), dtype=np.float32)
    w1u2[:D, :HID] = att_w1[:D, :]
    w1u2[D:, HID:] = att_w1[:D, :]
    # item-part weights tiled over all 8 member slots of a group
    w1i8 = np.tile(att_w1[D:, :], (1, 8))
    # block-diagonal w2 for scores
    w2blk = np.zeros((P, G8), dtype=np.float32)
    for j in range(G8):
        w2blk[j * HID : (j + 1) * HID, j] = att_w2[:, 0]
    pw_eg = pred_w1[: 2 * D, :]
    pw_it = pred_w1[2 * D :, :]
    b1r8 = np.tile(att_b1, 8).reshape(P, 1).astype(np.float32)
    return dict(
        w1u2=w1u2.astype(bf), w1i8=w1i8.astype(bf), w2blk=w2blk.astype(bf),
        pw_eg=pw_eg.astype(bf), pw_it=pw_it.astype(bf),
        pw2=pred_w2.astype(bf), b1r8=b1r8,
        ident=np.eye(P, dtype=np.float32).astype(bf),
        pb1=pred_b1.reshape(8, 1).astype(np.float32),
        pb2=np.asarray(pred_b2, np.float32).reshape(1, 1),
    )


def plan(member_lengths):
    """Per-core sort permutations + per-super-tile member widths (global max)."""
    m_len = np.asarray(member_lengths, np.int64).reshape(NC, BL)
    perms = [np.argsort(m_len[c], kind="stable") for c in range(NC)]
    widths = []
    for s in range(NS):
        wmax = 0
        for c in range(NC):
            seg = m_len[c][perms[c]][s * SW : (s + 1) * SW]
            wmax = max(wmax, int(seg.max()) + 1)
        widths.append(min(M, (wmax + 1) // 2 * 2))
    return perms, tuple(widths)


def make_in_maps(perms, widths, **inputs):
    import ml_dtypes

    bf = ml_dtypes.bfloat16
    st = _statics(
        np.asarray(inputs["att_w1"], np.float32),
        np.asarray(inputs["att_b1"], np.float32),
        np.asarray(inputs["att_w2"], np.float32),
        np.asarray(inputs["pred_w1"], np.float32),
        np.asarray(inputs["pred_b1"], np.float32),
        np.asarray(inputs["pred_w2"], np.float32),
        np.asarray(inputs["pred_b2"], np.float32),
    )

    def tile_major(x):
        # [BL] -> [P, NT]: column t holds samples t*128..t*128+127
        return np.ascontiguousarray(x.reshape(NT, P).T)

    def ids_plane(ids_sorted):
        # [BL, M] -> [P, sum(4*W_s)] truncated per super-tile, tile-major
        cols = []
        for s, w in enumerate(widths):
            seg = ids_sorted[s * SW : (s + 1) * SW, :w].reshape(ST, P, w)
            cols.append(seg.transpose(1, 0, 2).reshape(P, ST * w))
        return np.ascontiguousarray(np.concatenate(cols, axis=1))

    m_ids = np.asarray(inputs["member_ids"], np.int32).reshape(NC, BL, M)
    i_idx = np.asarray(inputs["item_inputs"], np.int32).reshape(NC, BL)
    g_idx = np.asarray(inputs["group_inputs"], np.int32).reshape(NC, BL)
    m_len = np.asarray(inputs["member_lengths"], np.float32).reshape(NC, BL)
    user_t = np.ascontiguousarray(np.asarray(inputs["user_table"], np.float32).astype(bf))
    item_t = np.ascontiguousarray(np.asarray(inputs["item_table"], np.float32).astype(bf))
    group_t = np.ascontiguousarray(np.asarray(inputs["group_table"], np.float32).astype(bf))

    in_maps = []
    for c in range(NC):
        p = perms[c]
        in_maps.append(
            {
                "m_ids": ids_plane(m_ids[c][p]),
                "i_idx": tile_major(i_idx[c][p]),
                "g_idx": tile_major(g_idx[c][p]),
                "m_len": tile_major(m_len[c][p]),
                "user_t": user_t,
                "item_t": item_t,
                "group_t": group_t,
                "w1u2": st["w1u2"],
                "w1i8": st["w1i8"],
                "w2blk": st["w2blk"],
                "pw_eg": st["pw_eg"],
                "pw_it": st["pw_it"],
                "pw2": st["pw2"],
                "b1r8": st["b1r8"],
                "pb1": st["pb1"],
                "pb2": st["pb2"],
                "ident": st["ident"],
            }
        )
    return in_maps


def get_nc(widths):
    if widths not in _CACHE:
        _CACHE[widths] = build_nc(widths)
    return _CACHE[widths]


def prepare(**inputs):
    perms, widths = plan(inputs["member_lengths"])
    nc = get_nc(widths)
    in_maps = make_in_maps(perms, widths, **inputs)
    return nc, in_maps, perms


def unsort(perms, results):
    outs = []
    for c in range(NC):
        o = np.empty((BL, 1), np.float32)
        o[perms[c]] = results[c]["out"]
        outs.append(o)
    return np.concatenate(outs, axis=0)


def kernel(**inputs):
    from concourse.bass_utils import run_bass_kernel_spmd

    nc, in_maps, perms = prepare(**inputs)
    res = run_bass_kernel_spmd(nc, in_maps, core_ids=list(range(NC)))
    return unsort(perms, res.results)
